# revision 2
# baseline (speedup 1.0000x reference)
"""Chamfer distance loss kernel for Trainium2 (8 NeuronCores, SPMD).

Problem: pred (4, 8192, 3) f32, target (4, 8192, 3) f32.
loss = mean_n min_m ||p_n - t_m||^2 + mean_m min_n ||p_n - t_m||^2

Sharding: 8 cores = 4 batches x 2 pred-row halves. Each core computes the
(4096 x 8192) block of the squared-distance matrix for its (batch, half):
  - row minima over all 8192 targets (exact dist1 contributions)
  - running column minima over its 4096 pred rows (partial dist2)
Host combines: sums row minima; collapses/mins the column-minima tiles.

Device pipeline per core:
  - PE: d = p2 + t2 - 2 p.t as K=18 (padded to 32) matmuls. Row chunks are
    packed into up to 4 concurrent 32-row PE groups via tile_position.
    PSUM tiles are [128, n, 512] (<=4 banks) double-buffered so the PE never
    stalls long enough for the HAM clock gate to re-throttle it.
  - ACT: evacuate PSUM -> SBUF with bf16 cast, one [128, n, 512] op per
    PSUM tile (1 elem/cycle/lane; ACT is the only engine that can move
    PSUM->SBUF at line rate).
  - DVE: colmin accumulate (tensor_tensor min, 2x bf16) + rowmin via a
    fold chain (tensor_tensor min halving, 2x bf16) + small tensor_reduce.
    DVE is the bottleneck engine (~1 cycle per matrix element total).
  - The colmin accumulator (cm, [128, 8192] bf16) ships to the host whole;
    the host collapses its partition axis (cheaper than the on-chip
    PE-transpose + 1x tensor_reduce tail).
Group schedule [2,4,4,4,4,4,4,4,1,1]: small first group + h-sliced colmin
ramps DVE early; tiny last group + h-sliced colmin shrinks the tail so the
cm DMA overlaps the last fold.
"""

import os
import sys

for _p in ("/opt/trn_rl_repo", "/opt/pypackages"):
    if _p not in sys.path:
        sys.path.insert(0, _p)

import numpy as np
import ml_dtypes

BF16 = ml_dtypes.bfloat16

B = 4
N = 8192  # pred points per batch
M = 8192  # target points per batch
HALF = N // 2  # pred rows per core = 4096
NCORES = 8
K = 18  # live contraction rows of the augmented matmul
PK = 32  # padded rows per PE row-group
RCHUNKS = HALF // 128  # 32 row chunks of 128 partitions
FT = 512  # matmul free-dim tile = one PSUM bank of fp32
HSL = M // FT  # 16 column slices per group

# chunk groups: (base_chunk, n_chunks); sum of n = 32
GROUPS = [(0, 2), (2, 4), (6, 4), (10, 4), (14, 4), (18, 4), (22, 4),
          (26, 4), (30, 1), (31, 1)]
NGRP = len(GROUPS)

_compiled = None


def _bf(x):
    return x.astype(BF16)


def _split3(x64):
    """3-way bf16 split of a float64 array; sum of parts ~ x to ~2^-24."""
    a = _bf(x64)
    r = x64 - a.astype(np.float64)
    b = _bf(r)
    r = r - b.astype(np.float64)
    c = _bf(r)
    return a, b, c


def _prep_lhs(p):
    """pred half (HALF, 3) f32 -> predAug (K, HALF) bf16."""
    p64 = p.astype(np.float64)
    ph = _bf(p)
    pl = _bf((p64 - ph.astype(np.float64)).astype(np.float32))
    p2 = (p64 * p64).sum(-1)
    a, b, c = _split3(p2)
    out = np.empty((K, p.shape[0]), dtype=BF16)
    out[0:3] = ph.T
    out[3:6] = pl.T
    out[6:9] = ph.T
    out[9:12] = pl.T
    out[12] = a
    out[13] = b
    out[14] = c
    out[15:18] = BF16(1.0)
    return out


def _prep_rhs(t):
    """target (M, 3) f32 -> targAug (K, M) bf16."""
    t64 = t.astype(np.float64)
    th = _bf(t)
    tl = _bf((t64 - th.astype(np.float64)).astype(np.float32))
    t2 = (t64 * t64).sum(-1)
    a, b, c = _split3(t2)
    out = np.empty((K, t.shape[0]), dtype=BF16)
    out[0:3] = (-2.0 * th.astype(np.float32)).astype(BF16).T
    out[3:6] = out[0:3]
    out[6:9] = (-2.0 * tl.astype(np.float32)).astype(BF16).T
    out[9:12] = out[6:9]
    out[12:15] = BF16(1.0)
    out[15] = a
    out[16] = b
    out[17] = c
    return out


def _build_program():
    import concourse.tile as tile
    from concourse import bacc, mybir

    nc = bacc.Bacc("TRN2", target_bir_lowering=False, debug=False, num_devices=NCORES)
    dt = mybir.dt
    Alu = mybir.AluOpType
    Ax = mybir.AxisListType

    pa_d = nc.dram_tensor(
        "pred_aug4", [128, NGRP * 128], dt.bfloat16, kind="ExternalInput"
    ).ap()
    ta_d = nc.dram_tensor("targ_aug4", [128, M], dt.bfloat16, kind="ExternalInput").ap()
    rm_d = nc.dram_tensor(
        "rowmins", [128, RCHUNKS], dt.float32, kind="ExternalOutput"
    ).ap()
    cm_d = nc.dram_tensor("cm_out", [128, M], dt.bfloat16, kind="ExternalOutput").ap()

    with tile.TileContext(nc) as tc:
        with (
            tc.tile_pool(name="consts", bufs=1) as consts,
            tc.tile_pool(name="dchunk", bufs=2) as dpool,
            tc.tile_pool(name="psum", bufs=2, space="PSUM") as psum,
        ):
            pa = consts.tile([128, NGRP * 128], dt.bfloat16)
            ta = consts.tile([128, M], dt.bfloat16)
            cm = consts.tile([128, M], dt.bfloat16)  # colmin accumulator
            rm = consts.tile([128, RCHUNKS], dt.float32)

            nc.sync.dma_start(pa[:], pa_d[:])
            # split the target DMA so the first matmuls start early
            for q in range(4):
                cs = slice(q * (M // 4), (q + 1) * (M // 4))
                nc.sync.dma_start(ta[:, cs], ta_d[:, cs])

            for g, (base, n) in enumerate(GROUPS):
                dc = dpool.tile([128, n * M], dt.bfloat16, tag="dc", name=f"dc_{g}")
                dcv = dc[:].rearrange("q (c w) -> q c w", c=n)
                first, last = g == 0, g == NGRP - 1
                for h in range(HSL):
                    hs = slice(h * FT, (h + 1) * FT)
                    pt = psum.tile([128, n, FT], dt.float32, tag="mm",
                                   name=f"pt_{g}_{h}")
                    for i in range(n):
                        nc.tensor.matmul(
                            pt[:, i, :],
                            lhsT=pa[32 * i:32 * i + PK, g * 128:(g + 1) * 128],
                            rhs=ta[32 * i:32 * i + PK, hs],
                            start=True,
                            stop=True,
                            tile_position=(32 * i, 0),
                        )
                    # one ACT evacuation per PSUM tile
                    nc.scalar.copy(dcv[:, :, hs], pt[:])
                    if first:
                        # ramp: colmin tracks the evacuation slice by slice
                        # so DVE starts immediately
                        nc.vector.tensor_copy(cm[:, hs], dcv[:, 0, hs])
                        for c in range(1, n):
                            nc.vector.tensor_tensor(
                                cm[:, hs], dcv[:, c, hs], cm[:, hs], op=Alu.min
                            )
                    elif last:
                        # tail: h-sliced colmin so the cm DMA can launch
                        # right after the final evacuation
                        nc.vector.tensor_tensor(
                            cm[:, hs], dcv[:, 0, hs], cm[:, hs], op=Alu.min
                        )
                if not (first or last):
                    for c in range(n):
                        nc.vector.tensor_tensor(
                            cm[:], dcv[:, c, :], cm[:], op=Alu.min
                        )
                if last:
                    nc.sync.dma_start(cm_d[:], cm[:])
                # rowmin fold chain (2x bf16); all n chunks per level
                w = M // 2
                while w >= 64:
                    nc.vector.tensor_tensor(
                        dcv[:, :, :w], dcv[:, :, :w], dcv[:, :, w:2 * w],
                        op=Alu.min,
                    )
                    w //= 2
                nc.vector.tensor_reduce(
                    rm[:, base:base + n], dcv[:, :, :64], axis=Ax.X, op=Alu.min
                )

            nc.sync.dma_start(rm_d[:], rm[:])

    nc.compile()
    return nc


def _get_program():
    global _compiled
    if _compiled is None:
        _compiled = _build_program()
    return _compiled


def make_in_maps(pred, target):
    """Build the per-core input dicts from full inputs."""
    pred = np.asarray(pred, dtype=np.float32)
    target = np.asarray(target, dtype=np.float32)
    in_maps = []
    for core in range(NCORES):
        b, half = divmod(core, 2)
        p = pred[b, half * HALF:(half + 1) * HALF]
        la = _prep_lhs(p)  # (K, HALF)
        ra = _prep_rhs(target[b])  # (K, M)

        # pack chunk groups into PE row bands:
        # group g col block holds chunks base..base+n-1 in bands 0..n-1
        pa4 = np.zeros((128, NGRP * 128), dtype=BF16)
        lc = la.reshape(K, RCHUNKS, 128)  # (K, chunk, col)
        for g, (base, n) in enumerate(GROUPS):
            for i in range(n):
                pa4[32 * i:32 * i + K, g * 128:(g + 1) * 128] = lc[:, base + i, :]
        # replicate targets into all 4 row bands, rows K..31 zero
        ta4 = np.zeros((128, M), dtype=BF16)
        for i in range(4):
            ta4[32 * i:32 * i + K] = ra
        in_maps.append({"pred_aug4": pa4, "targ_aug4": ta4})
    return in_maps


def combine(results):
    """Combine per-core outputs into the scalar loss."""
    d1 = 0.0
    d2 = 0.0
    for b in range(B):
        r0, r1 = results[2 * b], results[2 * b + 1]
        d1 += r0["rowmins"].astype(np.float64).sum()
        d1 += r1["rowmins"].astype(np.float64).sum()
        c0 = r0["cm_out"].astype(np.float32).min(axis=0)
        c1 = r1["cm_out"].astype(np.float32).min(axis=0)
        d2 += np.minimum(c0, c1).astype(np.float64).sum()
    loss = d1 / (B * N) + d2 / (B * M)
    return np.float32(loss)


def kernel(pred, target):
    from concourse.bass_utils import run_bass_kernel_spmd

    nc = _get_program()
    in_maps = make_in_maps(pred, target)
    res = run_bass_kernel_spmd(nc, in_maps, list(range(NCORES)))
    return np.asarray(combine(res.results))


# revision 5
# speedup vs baseline: 1.2815x; 1.2815x over previous
"""Chamfer distance loss kernel for Trainium2 (8 NeuronCores, SPMD).

Problem: pred (4, 8192, 3) f32, target (4, 8192, 3) f32.
loss = mean_n min_m ||p_n - t_m||^2 + mean_m min_n ||p_n - t_m||^2

Sharding: 8 cores = 4 batches x 2 pred-row halves. Each core computes the
(4096 x 8192) block of the squared-distance matrix for its (batch, half):
  - row minima over all 8192 targets (exact dist1 contributions)
  - running column minima over its 4096 pred rows (partial dist2)
Host combines: sums row minima; collapses/mins the column-minima tiles.

Device pipeline per core:
  - PE: d = p2 + t2 - 2 p.t as K=18 (padded to 32) matmuls. Row chunks are
    packed into up to 4 concurrent 32-row PE groups via tile_position.
    PSUM tiles are [128, n, 512] (<=4 banks) double-buffered so the PE never
    stalls long enough for the HAM clock gate to re-throttle it.
  - ACT: evacuate PSUM -> SBUF with bf16 cast, one [128, n, 512] op per
    PSUM tile (1 elem/cycle/lane; ACT is the only engine that can move
    PSUM->SBUF at line rate).
  - DVE: colmin accumulate (tensor_tensor min, 2x bf16) + rowmin via a
    fold chain (tensor_tensor min halving, 2x bf16) + small tensor_reduce.
    DVE is the bottleneck engine (~1 cycle per matrix element total).
  - The colmin accumulator (cm, [128, 8192] bf16) ships to the host whole;
    the host collapses its partition axis (cheaper than the on-chip
    PE-transpose + 1x tensor_reduce tail).
Group schedule [2,4,4,4,4,4,4,4,1,1]: small first group + h-sliced colmin
ramps DVE early; tiny last group + h-sliced colmin shrinks the tail so the
cm DMA overlaps the last fold.
"""

import os
import sys

for _p in ("/opt/trn_rl_repo", "/opt/pypackages"):
    if _p not in sys.path:
        sys.path.insert(0, _p)

import numpy as np
import ml_dtypes

BF16 = ml_dtypes.bfloat16

B = 4
N = 8192  # pred points per batch
M = 8192  # target points per batch
HALF = N // 2  # pred rows per core = 4096
NCORES = 8
K = 18  # live contraction rows of the augmented matmul
PK = 32  # padded rows per PE row-group
RCHUNKS = HALF // 128  # 32 row chunks of 128 partitions
FT = 512  # matmul free-dim tile = one PSUM bank of fp32
HSL = M // FT  # 16 column slices per group

# chunk groups: (base_chunk, n_chunks); sum of n = 32
GROUPS = [(0, 2), (2, 3), (5, 3), (8, 3), (11, 3), (14, 3), (17, 3),
          (20, 3), (23, 3), (26, 3), (29, 2), (31, 1)]
NGRP = len(GROUPS)

_compiled = None


def _bf(x):
    return x.astype(BF16)


def _split3(x64):
    """3-way bf16 split of a float64 array; sum of parts ~ x to ~2^-24."""
    a = _bf(x64)
    r = x64 - a.astype(np.float64)
    b = _bf(r)
    r = r - b.astype(np.float64)
    c = _bf(r)
    return a, b, c


def _prep_lhs(p):
    """pred half (HALF, 3) f32 -> predAug (K, HALF) bf16."""
    p64 = p.astype(np.float64)
    ph = _bf(p)
    pl = _bf((p64 - ph.astype(np.float64)).astype(np.float32))
    p2 = (p64 * p64).sum(-1)
    a, b, c = _split3(p2)
    out = np.empty((K, p.shape[0]), dtype=BF16)
    out[0:3] = ph.T
    out[3:6] = pl.T
    out[6:9] = ph.T
    out[9:12] = pl.T
    out[12] = a
    out[13] = b
    out[14] = c
    out[15:18] = BF16(1.0)
    return out


def _prep_rhs(t):
    """target (M, 3) f32 -> targAug (K, M) bf16."""
    t64 = t.astype(np.float64)
    th = _bf(t)
    tl = _bf((t64 - th.astype(np.float64)).astype(np.float32))
    t2 = (t64 * t64).sum(-1)
    a, b, c = _split3(t2)
    out = np.empty((K, t.shape[0]), dtype=BF16)
    out[0:3] = (-2.0 * th.astype(np.float32)).astype(BF16).T
    out[3:6] = out[0:3]
    out[6:9] = (-2.0 * tl.astype(np.float32)).astype(BF16).T
    out[9:12] = out[6:9]
    out[12:15] = BF16(1.0)
    out[15] = a
    out[16] = b
    out[17] = c
    return out


def _build_program():
    import concourse.tile as tile
    from concourse import bacc, mybir

    nc = bacc.Bacc("TRN2", target_bir_lowering=False, debug=False, num_devices=NCORES)
    dt = mybir.dt
    Alu = mybir.AluOpType
    Ax = mybir.AxisListType

    pa_d = nc.dram_tensor(
        "pred_aug4", [128, NGRP * 128], dt.bfloat16, kind="ExternalInput"
    ).ap()
    ta_d = nc.dram_tensor("targ_aug4", [128, M], dt.bfloat16, kind="ExternalInput").ap()
    rm_d = nc.dram_tensor(
        "rowmins", [128, RCHUNKS], dt.float32, kind="ExternalOutput"
    ).ap()
    cm_d = nc.dram_tensor("cm_out", [128, M], dt.bfloat16, kind="ExternalOutput").ap()

    with tile.TileContext(nc) as tc:
        with (
            tc.tile_pool(name="consts", bufs=1) as consts,
            tc.tile_pool(name="dchunk", bufs=3) as dpool,
            tc.tile_pool(name="psum", bufs=2, space="PSUM") as psum,
        ):
            pa = consts.tile([128, NGRP * 128], dt.bfloat16)
            ta = consts.tile([128, M], dt.bfloat16)
            cm = consts.tile([128, M], dt.bfloat16)  # colmin accumulator
            rm = consts.tile([128, RCHUNKS], dt.float32)

            nc.sync.dma_start(pa[:], pa_d[:])
            # split the target DMA so the first matmuls start early
            for q in range(4):
                cs = slice(q * (M // 4), (q + 1) * (M // 4))
                nc.sync.dma_start(ta[:, cs], ta_d[:, cs])

            for g, (base, n) in enumerate(GROUPS):
                dc = dpool.tile([128, n * M], dt.bfloat16, tag="dc", name=f"dc_{g}")
                dcv = dc[:].rearrange("q (c w) -> q c w", c=n)
                first, ramp2, last = g == 0, g == 1, g == NGRP - 1
                for h in range(HSL):
                    hs = slice(h * FT, (h + 1) * FT)
                    pt = psum.tile([128, n, FT], dt.float32, tag="mm",
                                   name=f"pt_{g}_{h}")
                    for i in range(n):
                        nc.tensor.matmul(
                            pt[:, i, :],
                            lhsT=pa[32 * i:32 * i + PK, g * 128:(g + 1) * 128],
                            rhs=ta[32 * i:32 * i + PK, hs],
                            start=True,
                            stop=True,
                            tile_position=(32 * i, 0),
                        )
                    # one ACT evacuation per PSUM tile
                    nc.scalar.copy(dcv[:, :, hs], pt[:])
                    if first:
                        # ramp: colmin tracks the evacuation slice by slice
                        # so DVE starts immediately
                        nc.vector.tensor_copy(cm[:, hs], dcv[:, 0, hs])
                        for c in range(1, n):
                            nc.vector.tensor_tensor(
                                cm[:, hs], dcv[:, c, hs], cm[:, hs], op=Alu.min
                            )
                    elif ramp2 and h % 2 == 1:
                        # second ramp group: colmin in 1024-wide slices so
                        # DVE is not stalled on the first full-group fill
                        h2 = slice((h - 1) * FT, (h + 1) * FT)
                        for c in range(n):
                            nc.vector.tensor_tensor(
                                cm[:, h2], dcv[:, c, h2], cm[:, h2], op=Alu.min
                            )
                    elif last:
                        # tail: h-sliced colmin so the cm DMA can launch
                        # right after the final evacuation
                        nc.vector.tensor_tensor(
                            cm[:, hs], dcv[:, 0, hs], cm[:, hs], op=Alu.min
                        )
                if not (first or ramp2 or last):
                    for c in range(n):
                        nc.vector.tensor_tensor(
                            cm[:], dcv[:, c, :], cm[:], op=Alu.min
                        )
                if last:
                    nc.sync.dma_start(cm_d[:], cm[:])
                # rowmin fold chain (2x bf16); all n chunks per level
                w = M // 2
                while w >= 64:
                    nc.vector.tensor_tensor(
                        dcv[:, :, :w], dcv[:, :, :w], dcv[:, :, w:2 * w],
                        op=Alu.min,
                    )
                    w //= 2
                nc.vector.tensor_reduce(
                    rm[:, base:base + n], dcv[:, :, :64], axis=Ax.X, op=Alu.min
                )

            nc.sync.dma_start(rm_d[:], rm[:])

    nc.compile()
    return nc


def _get_program():
    global _compiled
    if _compiled is None:
        _compiled = _build_program()
    return _compiled


def make_in_maps(pred, target):
    """Build the per-core input dicts from full inputs."""
    pred = np.asarray(pred, dtype=np.float32)
    target = np.asarray(target, dtype=np.float32)
    in_maps = []
    for core in range(NCORES):
        b, half = divmod(core, 2)
        p = pred[b, half * HALF:(half + 1) * HALF]
        la = _prep_lhs(p)  # (K, HALF)
        ra = _prep_rhs(target[b])  # (K, M)

        # pack chunk groups into PE row bands:
        # group g col block holds chunks base..base+n-1 in bands 0..n-1
        pa4 = np.zeros((128, NGRP * 128), dtype=BF16)
        lc = la.reshape(K, RCHUNKS, 128)  # (K, chunk, col)
        for g, (base, n) in enumerate(GROUPS):
            for i in range(n):
                pa4[32 * i:32 * i + K, g * 128:(g + 1) * 128] = lc[:, base + i, :]
        # replicate targets into all 4 row bands, rows K..31 zero
        ta4 = np.zeros((128, M), dtype=BF16)
        for i in range(4):
            ta4[32 * i:32 * i + K] = ra
        in_maps.append({"pred_aug4": pa4, "targ_aug4": ta4})
    return in_maps


def combine(results):
    """Combine per-core outputs into the scalar loss."""
    d1 = 0.0
    d2 = 0.0
    for b in range(B):
        r0, r1 = results[2 * b], results[2 * b + 1]
        d1 += r0["rowmins"].astype(np.float64).sum()
        d1 += r1["rowmins"].astype(np.float64).sum()
        c0 = r0["cm_out"].astype(np.float32).min(axis=0)
        c1 = r1["cm_out"].astype(np.float32).min(axis=0)
        d2 += np.minimum(c0, c1).astype(np.float64).sum()
    loss = d1 / (B * N) + d2 / (B * M)
    return np.float32(loss)


def kernel(pred, target):
    from concourse.bass_utils import run_bass_kernel_spmd

    nc = _get_program()
    in_maps = make_in_maps(pred, target)
    res = run_bass_kernel_spmd(nc, in_maps, list(range(NCORES)))
    return np.asarray(combine(res.results))


# revision 8
# speedup vs baseline: 1.3057x; 1.0189x over previous
"""Chamfer distance loss kernel for Trainium2 (8 NeuronCores, SPMD).

Problem: pred (4, 8192, 3) f32, target (4, 8192, 3) f32.
loss = mean_n min_m ||p_n - t_m||^2 + mean_m min_n ||p_n - t_m||^2

Sharding: 8 cores = 4 batches x 2 pred-row halves. Each core computes the
(4096 x 8192) block of the squared-distance matrix for its (batch, half):
  - row minima over all 8192 targets (exact dist1 contributions)
  - running column minima over its 4096 pred rows (partial dist2)
Host combines: sums row minima; collapses/mins the column-minima tiles.

Device pipeline per core:
  - PE: d = p2 + t2 - 2 p.t as K=18 (padded to 32) matmuls. Row chunks are
    packed into up to 4 concurrent 32-row PE groups via tile_position.
    PSUM tiles are [128, n, 512] (<=4 banks) double-buffered so the PE never
    stalls long enough for the HAM clock gate to re-throttle it.
  - ACT: evacuate PSUM -> SBUF with bf16 cast, one [128, n, 512] op per
    PSUM tile (1 elem/cycle/lane; ACT is the only engine that can move
    PSUM->SBUF at line rate).
  - DVE: colmin accumulate (tensor_tensor min, 2x bf16) + rowmin via a
    fold chain (tensor_tensor min halving, 2x bf16) + small tensor_reduce.
    DVE is the bottleneck engine (~1 cycle per matrix element total).
  - The colmin accumulator (cm, [128, 8192] bf16) ships to the host whole;
    the host collapses its partition axis (cheaper than the on-chip
    PE-transpose + 1x tensor_reduce tail).
Group schedule [2,4,4,4,4,4,4,4,1,1]: small first group + h-sliced colmin
ramps DVE early; tiny last group + h-sliced colmin shrinks the tail so the
cm DMA overlaps the last fold.
"""

import os
import sys

for _p in ("/opt/trn_rl_repo", "/opt/pypackages"):
    if _p not in sys.path:
        sys.path.insert(0, _p)

import numpy as np
import ml_dtypes

BF16 = ml_dtypes.bfloat16

B = 4
N = 8192  # pred points per batch
M = 8192  # target points per batch
HALF = N // 2  # pred rows per core = 4096
NCORES = 8
K = 18  # live contraction rows of the augmented matmul
PK = 32  # padded rows per PE row-group
RCHUNKS = HALF // 128  # 32 row chunks of 128 partitions
FT = 512  # matmul free-dim tile = one PSUM bank of fp32
HSL = M // FT  # 16 column slices per group

# chunk groups: (base_chunk, n_chunks); sum of n = 32
GROUPS = [(0, 2), (2, 3), (5, 3), (8, 3), (11, 3), (14, 3), (17, 3),
          (20, 3), (23, 3), (26, 3), (29, 2), (31, 1)]
NGRP = len(GROUPS)

_compiled = None


def _bf(x):
    return x.astype(BF16)


def _split3(x64):
    """3-way bf16 split of a float64 array; sum of parts ~ x to ~2^-24."""
    a = _bf(x64)
    r = x64 - a.astype(np.float64)
    b = _bf(r)
    r = r - b.astype(np.float64)
    c = _bf(r)
    return a, b, c


def _prep_lhs(p):
    """pred half (HALF, 3) f32 -> predAug (K, HALF) bf16."""
    p64 = p.astype(np.float64)
    ph = _bf(p)
    pl = _bf((p64 - ph.astype(np.float64)).astype(np.float32))
    p2 = (p64 * p64).sum(-1)
    a, b, c = _split3(p2)
    out = np.empty((K, p.shape[0]), dtype=BF16)
    out[0:3] = ph.T
    out[3:6] = pl.T
    out[6:9] = ph.T
    out[9:12] = pl.T
    out[12] = a
    out[13] = b
    out[14] = c
    out[15:18] = BF16(1.0)
    return out


def _prep_rhs(t):
    """target (M, 3) f32 -> targAug (K, M) bf16."""
    t64 = t.astype(np.float64)
    th = _bf(t)
    tl = _bf((t64 - th.astype(np.float64)).astype(np.float32))
    t2 = (t64 * t64).sum(-1)
    a, b, c = _split3(t2)
    out = np.empty((K, t.shape[0]), dtype=BF16)
    out[0:3] = (-2.0 * th.astype(np.float32)).astype(BF16).T
    out[3:6] = out[0:3]
    out[6:9] = (-2.0 * tl.astype(np.float32)).astype(BF16).T
    out[9:12] = out[6:9]
    out[12:15] = BF16(1.0)
    out[15] = a
    out[16] = b
    out[17] = c
    return out


def _build_program():
    import concourse.tile as tile
    from concourse import bacc, mybir

    nc = bacc.Bacc("TRN2", target_bir_lowering=False, debug=False, num_devices=NCORES)
    dt = mybir.dt
    Alu = mybir.AluOpType
    Ax = mybir.AxisListType

    pa_d = nc.dram_tensor(
        "pred_aug4", [128, NGRP * 128], dt.bfloat16, kind="ExternalInput"
    ).ap()
    ta_d = nc.dram_tensor("targ_aug4", [128, M], dt.bfloat16, kind="ExternalInput").ap()
    rm_d = nc.dram_tensor(
        "rowmins", [128, RCHUNKS], dt.float32, kind="ExternalOutput"
    ).ap()
    cm_d = nc.dram_tensor("cm_out", [128, M], dt.bfloat16, kind="ExternalOutput").ap()

    with tile.TileContext(nc) as tc:
        with (
            tc.tile_pool(name="consts", bufs=1) as consts,
            tc.tile_pool(name="dchunk", bufs=3) as dpool,
            tc.tile_pool(name="psum", bufs=2, space="PSUM") as psum,
        ):
            pa = consts.tile([128, NGRP * 128], dt.bfloat16)
            ta = consts.tile([128, M], dt.bfloat16)
            cm = consts.tile([128, M], dt.bfloat16)  # colmin accumulator
            rm = consts.tile([128, RCHUNKS], dt.float32)

            # DMA order: first target slice + weights first so the first
            # matmuls (needing ta[:, :512] and pa[:, :128]) start ASAP
            nc.sync.dma_start(ta[:, 0:1024], ta_d[:, 0:1024])
            nc.sync.dma_start(pa[:], pa_d[:])
            for q in range(1, 8):
                cs = slice(q * 1024, (q + 1) * 1024)
                nc.sync.dma_start(ta[:, cs], ta_d[:, cs])

            for g, (base, n) in enumerate(GROUPS):
                dc = dpool.tile([128, n * M], dt.bfloat16, tag="dc", name=f"dc_{g}")
                dcv = dc[:].rearrange("q (c w) -> q c w", c=n)
                first, last = g == 0, g == NGRP - 1
                ramp = g in (1, 2)
                for h in range(HSL):
                    hs = slice(h * FT, (h + 1) * FT)
                    pt = psum.tile([128, n, FT], dt.float32, tag="mm",
                                   name=f"pt_{g}_{h}")
                    for i in range(n):
                        nc.tensor.matmul(
                            pt[:, i, :],
                            lhsT=pa[32 * i:32 * i + PK, g * 128:(g + 1) * 128],
                            rhs=ta[32 * i:32 * i + PK, hs],
                            start=True,
                            stop=True,
                            tile_position=(32 * i, 0),
                        )
                    # one ACT evacuation per PSUM tile
                    nc.scalar.copy(dcv[:, :, hs], pt[:])
                    if first and h % 2 == 1:
                        # ramp: colmin tracks the evacuation 1024 cols at a
                        # time so DVE starts immediately
                        h2 = slice((h - 1) * FT, (h + 1) * FT)
                        nc.vector.tensor_copy(cm[:, h2], dcv[:, 0, h2])
                        for c in range(1, n):
                            nc.vector.tensor_tensor(
                                cm[:, h2], dcv[:, c, h2], cm[:, h2], op=Alu.min
                            )
                    elif ramp and h % 4 == 3:
                        # early groups: colmin in 2048-wide slices so DVE is
                        # not stalled on a full-group fill during the ramp
                        h4 = slice((h - 3) * FT, (h + 1) * FT)
                        for c in range(n):
                            nc.vector.tensor_tensor(
                                cm[:, h4], dcv[:, c, h4], cm[:, h4], op=Alu.min
                            )
                    elif last and h % 2 == 1:
                        # tail: sliced colmin + sliced cm DMA so the output
                        # leaves right behind the final evacuations
                        h2 = slice((h - 1) * FT, (h + 1) * FT)
                        nc.vector.tensor_tensor(
                            cm[:, h2], dcv[:, 0, h2], cm[:, h2], op=Alu.min
                        )
                        if h % 4 == 3:
                            h4 = slice((h - 3) * FT, (h + 1) * FT)
                            nc.sync.dma_start(cm_d[:, h4], cm[:, h4])
                if not (first or ramp or last):
                    for c in range(n):
                        nc.vector.tensor_tensor(
                            cm[:], dcv[:, c, :], cm[:], op=Alu.min
                        )
                # rowmin fold chain (2x bf16); all n chunks per level
                w = M // 2
                while w >= 64:
                    nc.vector.tensor_tensor(
                        dcv[:, :, :w], dcv[:, :, :w], dcv[:, :, w:2 * w],
                        op=Alu.min,
                    )
                    w //= 2
                nc.vector.tensor_reduce(
                    rm[:, base:base + n], dcv[:, :, :64], axis=Ax.X, op=Alu.min
                )

            # rm goes out on the scalar engine's DMA queue, concurrent with
            # the tail cm slices on the sync queue
            nc.scalar.dma_start(rm_d[:], rm[:])

    nc.compile()
    return nc


def _get_program():
    global _compiled
    if _compiled is None:
        _compiled = _build_program()
    return _compiled


def make_in_maps(pred, target):
    """Build the per-core input dicts from full inputs."""
    pred = np.asarray(pred, dtype=np.float32)
    target = np.asarray(target, dtype=np.float32)
    in_maps = []
    for core in range(NCORES):
        b, half = divmod(core, 2)
        p = pred[b, half * HALF:(half + 1) * HALF]
        la = _prep_lhs(p)  # (K, HALF)
        ra = _prep_rhs(target[b])  # (K, M)

        # pack chunk groups into PE row bands:
        # group g col block holds chunks base..base+n-1 in bands 0..n-1
        pa4 = np.zeros((128, NGRP * 128), dtype=BF16)
        lc = la.reshape(K, RCHUNKS, 128)  # (K, chunk, col)
        for g, (base, n) in enumerate(GROUPS):
            for i in range(n):
                pa4[32 * i:32 * i + K, g * 128:(g + 1) * 128] = lc[:, base + i, :]
        # replicate targets into all 4 row bands, rows K..31 zero
        ta4 = np.zeros((128, M), dtype=BF16)
        for i in range(4):
            ta4[32 * i:32 * i + K] = ra
        in_maps.append({"pred_aug4": pa4, "targ_aug4": ta4})
    return in_maps


def combine(results):
    """Combine per-core outputs into the scalar loss."""
    d1 = 0.0
    d2 = 0.0
    for b in range(B):
        r0, r1 = results[2 * b], results[2 * b + 1]
        d1 += r0["rowmins"].astype(np.float64).sum()
        d1 += r1["rowmins"].astype(np.float64).sum()
        c0 = r0["cm_out"].astype(np.float32).min(axis=0)
        c1 = r1["cm_out"].astype(np.float32).min(axis=0)
        d2 += np.minimum(c0, c1).astype(np.float64).sum()
    loss = d1 / (B * N) + d2 / (B * M)
    return np.float32(loss)


def kernel(pred, target):
    from concourse.bass_utils import run_bass_kernel_spmd

    nc = _get_program()
    in_maps = make_in_maps(pred, target)
    res = run_bass_kernel_spmd(nc, in_maps, list(range(NCORES)))
    return np.asarray(combine(res.results))
